# revision 10
# baseline (speedup 1.0000x reference)
"""CRQVAE (encoder MLP + 4-level residual VQ) on 8 TRN2 NeuronCores.

Data-parallel: batch N=131072 sharded 8 ways (16384 rows/core); encoder
weights + codebooks replicated. Only the scalar rq_loss needs a cross-core
reduction, done on the host from tiny per-core partials.

Device-side design (per core, feature-major activations):
  - Host passes x already transposed: xT [768, 16384].
  - 32 batch tiles of 512 columns. Per tile:
      L1/L2/L3 fp32 matmuls (weights stationary, activations moving,
      feature-major) with ACT relu+bias PSUM->SBUF drains.
  - RVQ per level, with scores scaled by LARGE=2^60:
      batch-major scores  S[n,k] = LARGE*(2 r.c - |c|^2)   (for row max m)
      DVE max-reduce -> m[128,1] per n-chunk; PE-transpose m-cols to a row;
      feature-major scores S'[k,n] = LARGE*(2 r.c - |c|^2) - m  (extra
      contraction rows carry -m), bit-identical to batch scores by
      construction, so relu(S' + 1) is an *exact* one-hot of the argmax.
      q/idx gather via onehot matmul against [cb | 0 | iota].
      residual update r -= q on DVE.
  - Outputs: x_qT, zT (feature-major), idx rows (f32), m values (for the
    host-side loss telescope  |r_{l+1}|^2 = |z|^2 - sum_{j<=l} m_j/LARGE).
"""
import os
import sys

sys.path.insert(0, "/opt/trn_rl_repo")

import numpy as np

import concourse.bass as bass
import concourse.tile as tile
from concourse import bacc, mybir
from concourse import bass_utils

F32 = mybir.dt.float32
BF16 = mybir.dt.bfloat16
AF = mybir.ActivationFunctionType

N_CORES = 8
N_TOTAL = 131072
N_PER_CORE = N_TOTAL // N_CORES        # 16384
B = 512                                # batch columns per tile
N_TILES = N_PER_CORE // B              # 32
D0, D1, D2, E = 768, 512, 256, 64
K = 256                                # codebook size
L = 4                                  # rvq levels
LARGE = float(2.0 ** 60)
BETA = 0.25


def build_kernel(n_tiles=N_TILES):
    nc = bacc.Bacc("TRN2", target_bir_lowering=False, debug=False)
    n = n_tiles * B

    # ---------------- DRAM parameters ----------------
    xT_d = nc.dram_tensor("xT", [D0, n], F32, kind="ExternalInput")
    w0_d = nc.dram_tensor("w0", [D0, D1], F32, kind="ExternalInput")
    w1_d = nc.dram_tensor("w1", [D1, D2], F32, kind="ExternalInput")
    w2_d = nc.dram_tensor("w2", [D2, E + 1], F32, kind="ExternalInput")  # [W2|0]
    b0_d = nc.dram_tensor("b0", [128, D1 // 128], F32, kind="ExternalInput")
    b1_d = nc.dram_tensor("b1", [128, D2 // 128], F32, kind="ExternalInput")
    b2_d = nc.dram_tensor("b2", [E + 1, 1], F32, kind="ExternalInput")   # [b2;1]
    cba_d = nc.dram_tensor("cba", [L, E + 1, K], F32, kind="ExternalInput")
    cbx_d = nc.dram_tensor("cbx", [3, L, K, 97], mybir.dt.bfloat16, kind="ExternalInput")
    eyeb_d = nc.dram_tensor("eyeb", [128, 128], mybir.dt.bfloat16, kind="ExternalInput")

    xqT_d = nc.dram_tensor("xqT", [E, n], F32, kind="ExternalOutput")
    zT_d = nc.dram_tensor("zT", [E, n], F32, kind="ExternalOutput")
    idx_d = nc.dram_tensor("idx", [L, n], F32, kind="ExternalOutput")
    mv_d = nc.dram_tensor("mv", [128, n_tiles * L * 4], F32, kind="ExternalOutput")

    KC1, KC2, KC3 = D0 // 128, D1 // 128, D2 // 128   # 6, 4, 2
    JC1, JC2 = D1 // 128, D2 // 128                   # 4, 2
    NC = B // 128                                     # 4 n-chunks per tile
    KCC = K // 128                                    # 2 code chunks

    with tile.TileContext(nc) as tc:
        with (
            tc.tile_pool(name="const", bufs=1) as cp,
            tc.tile_pool(name="xin", bufs=3) as xp,
            tc.tile_pool(name="act", bufs=2) as ap,
            tc.tile_pool(name="rres", bufs=3) as rp,
            tc.tile_pool(name="oh", bufs=2) as ohp,
            tc.tile_pool(name="stage", bufs=3) as stp,
            tc.tile_pool(name="pmlp", bufs=2, space="PSUM") as pmlp,
            tc.tile_pool(name="pz", bufs=1, space="PSUM") as pz,
            tc.tile_pool(name="pbs", bufs=1, space="PSUM") as pbs,
            tc.tile_pool(name="pfs", bufs=1, space="PSUM") as pfs,
            tc.tile_pool(name="pq", bufs=1, space="PSUM") as pq,
        ):
            # ---- constants (loaded once) ----
            w0 = cp.tile([128, KC1, D1], F32, tag="w0")
            w1 = cp.tile([128, KC2, D2], F32, tag="w1")
            w2 = cp.tile([128, KC3, E + 1], F32, tag="w2")
            b0c = cp.tile([128, JC1], F32, tag="b0")
            b1c = cp.tile([128, JC2], F32, tag="b1")
            b2c = cp.tile([E + 1, 1], F32, tag="b2")
            eyeb = cp.tile([128, 128], BF16, tag="eyeb")
            cba = [cp.tile([E + 1, K], F32, tag=f"cba{l}", name=f"cba{l}") for l in range(L)]
            cbx = [cp.tile([128, KCC, 3, 97], BF16, tag=f"cbx{l}", name=f"cbx{l}") for l in range(L)]

            for c in range(KC1):
                nc.sync.dma_start(w0[:, c, :], w0_d[c * 128:(c + 1) * 128, :])
            for c in range(KC2):
                nc.sync.dma_start(w1[:, c, :], w1_d[c * 128:(c + 1) * 128, :])
            for c in range(KC3):
                nc.sync.dma_start(w2[:, c, :], w2_d[c * 128:(c + 1) * 128, :])
            nc.sync.dma_start(b0c[:], b0_d[:])
            nc.sync.dma_start(b1c[:], b1_d[:])
            nc.sync.dma_start(b2c[:], b2_d[:])
            nc.sync.dma_start(eyeb[:], eyeb_d[:])
            for l in range(L):
                nc.sync.dma_start(cba[l][:], cba_d[l])
                for t in range(3):
                    for kc in range(KCC):
                        nc.sync.dma_start(cbx[l][:, kc, t, :],
                                          cbx_d[t, l, kc * 128:(kc + 1) * 128, :])

            for i in range(n_tiles):
                sl = slice(i * B, (i + 1) * B)

                # ---- load xT tile ----
                xt = xp.tile([128, KC1, B], F32, tag="x")
                for c in range(KC1):
                    nc.sync.dma_start(xt[:, c, :], xT_d[c * 128:(c + 1) * 128, sl])

                # ---- L1: h1 = relu(W0.T @ xT + b0) ----
                h1 = ap.tile([128, JC1, B], F32, tag="h1")
                for j in range(JC1):
                    pm = pmlp.tile([128, B], F32, tag="pm")
                    for c in range(KC1):
                        nc.tensor.matmul(pm[:], w0[:, c, j * 128:(j + 1) * 128],
                                         xt[:, c, :],
                                         start=(c == 0), stop=(c == KC1 - 1))
                    nc.scalar.activation(h1[:, j, :], pm[:], AF.Relu,
                                         bias=b0c[:, j:j + 1], scale=1.0)

                # ---- L2 ----
                h2 = ap.tile([128, JC2, B], F32, tag="h2")
                for j in range(JC2):
                    pm = pmlp.tile([128, B], F32, tag="pm")
                    for c in range(KC2):
                        nc.tensor.matmul(pm[:], w1[:, c, j * 128:(j + 1) * 128],
                                         h1[:, c, :],
                                         start=(c == 0), stop=(c == KC2 - 1))
                    nc.scalar.activation(h2[:, j, :], pm[:], AF.Relu,
                                         bias=b1c[:, j:j + 1], scale=1.0)

                # ---- L3: z''T = [z; 1; m-row] ----
                pzt = pz.tile([E + 1, B], F32, tag="pz")
                for c in range(KC3):
                    nc.tensor.matmul(pzt[:], w2[:, c, :], h2[:, c, :],
                                     start=(c == 0), stop=(c == KC3 - 1))
                r0 = rp.tile([E + 1, B], F32, tag="r")
                # rows 0..63 = z + b2 ; row 64 = 0 + 1 = 1
                nc.scalar.activation(r0[0:E + 1, :], pzt[:], AF.Identity,
                                     bias=b2c[:], scale=1.0)
                nc.sync.dma_start(zT_d[:, sl], r0[0:E, :])

                mst = stp.tile([128, L * NC], F32, tag="mst")

                r_cur = r0
                for l in range(L):
                    # ---- batch-major scores s = 2 r.c - |c|^2 -> row max ----
                    pb = pbs.tile([128, NC, K], F32, tag="pb")
                    for c in range(NC):
                        nc.tensor.matmul(pb[:, c, :],
                                         r_cur[0:E + 1, c * 128:(c + 1) * 128],
                                         cba[l][:], start=True, stop=True)
                    nc.vector.tensor_reduce(mst[:, l * NC:(l + 1) * NC], pb[:],
                                            axis=mybir.AxisListType.X,
                                            op=mybir.AluOpType.max)

                    # ---- exact one-hot: (s >= rowmax), bf16 ----
                    oh = ohp.tile([128, NC, K], BF16, tag="oh")
                    for c in range(NC):
                        nc.vector.tensor_scalar(
                            oh[:, c, :], pb[:, c, :],
                            scalar1=mst[:, l * NC + c:l * NC + c + 1],
                            scalar2=None, op0=mybir.AluOpType.is_ge)

                    # ---- transpose one-hot to code-major (PE, bf16) ----
                    pt = pfs.tile([128, KCC, B], BF16, tag="pt")
                    for c in range(NC):
                        for kc in range(KCC):
                            nc.tensor.transpose(
                                pt[:, kc, c * 128:(c + 1) * 128],
                                oh[:, c, kc * 128:(kc + 1) * 128], eyeb[:])
                    ohT = ohp.tile([128, KCC, B], BF16, tag="ohT")
                    if l % 2 == 0:
                        nc.scalar.copy(ohT[:], pt[:])
                    else:
                        nc.vector.tensor_copy(ohT[:], pt[:])

                    # ---- q / idx gather: 3-term bf16 split of cb (exact) ----
                    pqt = pq.tile([97, B], F32, tag="pq")
                    nmm = 3 * KCC
                    for t in range(3):
                        for kc in range(KCC):
                            nc.tensor.matmul(pqt[:], cbx[l][:, kc, t, :],
                                             ohT[:, kc, :],
                                             start=(t == 0 and kc == 0),
                                             stop=(t == 2 and kc == KCC - 1))
                    ixl = stp.tile([1, B], F32, tag=f"ist{l}", name=f"ist{l}")
                    if l % 2 == 0:
                        nc.vector.tensor_copy(ixl[:], pqt[96:97, :])
                    else:
                        nc.scalar.copy(ixl[:], pqt[96:97, :])
                    nc.gpsimd.dma_start(idx_d[l:l + 1, sl], ixl[:])

                    # ---- residual update (rows 0..64; row64: 1-0=1) ----
                    r_nxt = rp.tile([E + 1, B], F32, tag="r")
                    nc.vector.tensor_tensor(r_nxt[0:E + 1, :], r_cur[0:E + 1, :],
                                            pqt[0:E + 1, :],
                                            op=mybir.AluOpType.subtract)
                    r_cur = r_nxt

                # ---- x_q = z - r4 ----
                xq = stp.tile([E, B], F32, tag="xq")
                nc.vector.tensor_tensor(xq[:], r0[0:E, :], r_cur[0:E, :],
                                        op=mybir.AluOpType.subtract)
                nc.gpsimd.dma_start(xqT_d[:, sl], xq[:])
                nc.gpsimd.dma_start(mv_d[:, i * L * NC:(i + 1) * L * NC], mst[:])

    nc.compile()
    return nc


def _host_prep(x, W0, b0, W1, b1, W2, b2, codebooks):
    """Build per-core input maps."""
    cb = np.asarray(codebooks, np.float32)                    # [4,256,64]
    cn = (cb.astype(np.float64) ** 2).sum(-1)
    cn32 = ((cb ** 2).sum(-1)).astype(np.float32)             # fp32 |c|^2

    cba = np.zeros((L, E + 1, K), np.float32)
    cba[:, :E, :] = np.transpose(cb, (0, 2, 1)) * 2.0
    cba[:, E, :] = -cn32

    import ml_dtypes
    cbf = np.zeros((L, K, 97), np.float32)
    cbf[:, :, :E] = cb
    cbf[:, :, 96] = np.arange(K, dtype=np.float32)[None, :]
    h = cbf.astype(ml_dtypes.bfloat16)
    r1 = cbf - h.astype(np.float32)
    m_ = r1.astype(ml_dtypes.bfloat16)
    r2 = r1 - m_.astype(np.float32)
    lo = r2.astype(ml_dtypes.bfloat16)
    cbx = np.stack([h, m_, lo])                      # [3, L, K, 97] bf16
    eyeb = np.eye(128, dtype=ml_dtypes.bfloat16)

    w2a = np.zeros((D2, E + 1), np.float32)
    w2a[:, :E] = np.asarray(W2, np.float32)
    b2a = np.zeros((E + 1, 1), np.float32)
    b2a[:E, 0] = np.asarray(b2, np.float32)
    b2a[E, 0] = 1.0

    common = {
        "w0": np.ascontiguousarray(np.asarray(W0, np.float32)),
        "w1": np.ascontiguousarray(np.asarray(W1, np.float32)),
        "w2": w2a,
        "b0": np.ascontiguousarray(np.asarray(b0, np.float32).reshape(D1 // 128, 128).T),
        "b1": np.ascontiguousarray(np.asarray(b1, np.float32).reshape(D2 // 128, 128).T),
        "b2": b2a,
        "cba": cba, "cbx": cbx, "eyeb": eyeb,
    }

    x = np.asarray(x, np.float32)
    in_maps = []
    for c in range(N_CORES):
        xs = x[c * N_PER_CORE:(c + 1) * N_PER_CORE]
        m = dict(common)
        m["xT"] = np.ascontiguousarray(xs.T)
        in_maps.append(m)
    return in_maps


_NC_CACHE = {}


class _Res:
    def __init__(self, results):
        self.results = results


def _run_pjrt(nc, in_maps, n_cores, time_reps=0):
    """Execute the compiled Bass graph on n_cores via PJRT (axon), optionally
    timing warm repeat executions with device-resident inputs."""
    import time as _time
    import jax
    from jax.sharding import Mesh, PartitionSpec
    from jax.experimental.shard_map import shard_map
    from concourse import bass2jax, mybir as _mb
    from concourse.bass2jax import _bass_exec_p, install_neuronx_cc_hook

    install_neuronx_cc_hook()
    partition_name = nc.partition_id_tensor.name if nc.partition_id_tensor else None

    in_names, out_names, out_avals, zero_outs = [], [], [], []
    for alloc in nc.m.functions[0].allocations:
        if not isinstance(alloc, _mb.MemoryLocationSet):
            continue
        name = alloc.memorylocations[0].name
        if alloc.kind == "ExternalInput":
            if name != partition_name:
                in_names.append(name)
        elif alloc.kind == "ExternalOutput":
            shape = tuple(alloc.tensor_shape)
            dtype = _mb.dt.np(alloc.dtype)
            out_names.append(name)
            out_avals.append(jax.core.ShapedArray(shape, dtype))
            zero_outs.append(np.zeros(shape, dtype))
    n_params = len(in_names)
    n_outs = len(out_avals)
    all_in_names = list(in_names) + list(out_names)
    if partition_name is not None:
        all_in_names.append(partition_name)

    def _body(*args):
        operands = list(args)
        if partition_name is not None:
            operands.append(bass2jax.partition_id_tensor())
        outs = _bass_exec_p.bind(
            *operands,
            out_avals=tuple(out_avals),
            in_names=tuple(all_in_names),
            out_names=tuple(out_names),
            lowering_input_output_aliases=(),
            sim_require_finite=True,
            sim_require_nnan=True,
            nc=nc,
        )
        return tuple(outs)

    devices = jax.devices()[:n_cores]
    mesh = Mesh(np.asarray(devices), ("core",))
    in_specs = (PartitionSpec("core"),) * (n_params + n_outs)
    out_specs = (PartitionSpec("core"),) * n_outs
    sharded = jax.jit(
        shard_map(_body, mesh=mesh, in_specs=in_specs, out_specs=out_specs,
                  check_rep=False),
        keep_unused=True,
    )
    concat_in = [
        np.concatenate([np.asarray(in_maps[c][nm]) for c in range(n_cores)], axis=0)
        for nm in in_names
    ]
    concat_zeros = [np.zeros((n_cores * z.shape[0], *z.shape[1:]), z.dtype)
                    for z in zero_outs]
    args = concat_in + concat_zeros
    out_arrs = sharded(*args)
    jax.block_until_ready(out_arrs)

    if time_reps > 0:
        sh = jax.sharding.NamedSharding(mesh, PartitionSpec("core"))
        dev_args = [jax.device_put(a, sh) for a in args]
        jax.block_until_ready(dev_args)
        jax.block_until_ready(sharded(*dev_args))  # warm
        t0 = _time.perf_counter()
        o = None
        for _ in range(time_reps):
            o = sharded(*dev_args)
        jax.block_until_ready(o)
        dt = (_time.perf_counter() - t0) / time_reps
        ns = int(dt * 1e9)
        _NC_CACHE["exec_time_ns"] = ns
        print(f"HW exec time: {ns} ns   (warm wall avg over {time_reps} reps)")

    results = [
        {name: np.asarray(out_arrs[i]).reshape(n_cores, *out_avals[i].shape)[c]
         for i, name in enumerate(out_names)}
        for c in range(n_cores)
    ]
    return _Res(results)


def kernel(x, W0, b0, W1, b1, W2, b2, codebooks):
    if "nc" not in _NC_CACHE:
        _NC_CACHE["nc"] = build_kernel()
    nc = _NC_CACHE["nc"]

    in_maps = _host_prep(x, W0, b0, W1, b1, W2, b2, codebooks)
    reps = int(os.environ.get("KERNEL_TIME_REPS", "0"))
    res = _run_pjrt(nc, in_maps, N_CORES, time_reps=reps)

    x_q = np.empty((N_TOTAL, E), np.float32)
    codes = np.empty((L, N_TOTAL), np.int32)
    total_znorm = 0.0
    msums = np.zeros(L, dtype=np.float64)
    for c in range(N_CORES):
        out = res.results[c]
        x_q[c * N_PER_CORE:(c + 1) * N_PER_CORE] = out["xqT"].T
        codes[:, c * N_PER_CORE:(c + 1) * N_PER_CORE] = \
            np.rint(out["idx"]).astype(np.int32)
        total_znorm += float((out["zT"].astype(np.float64) ** 2).sum())
        mv = out["mv"].astype(np.float64).reshape(128, N_TILES, L, 4)
        msums += mv.sum(axis=(0, 1, 3))

    # loss telescope: sum_n |r_{l+1}|^2 = sum|z|^2 - sum_{j<=l} msum_j
    denom = float(N_TOTAL * E)
    losses = []
    run = total_znorm
    for l in range(L):
        run -= msums[l]
        losses.append((1.0 + BETA) * run / denom)
    rq_loss = np.float32(np.mean(losses))

    return x_q, rq_loss, codes


if __name__ == "__main__":
    import reference
    inputs = reference.setup_inputs()
    inputs = {k: np.asarray(v) for k, v in inputs.items()}
    out = kernel(**inputs)
    print("x_q", out[0].shape, "loss", out[1], "codes", out[2].shape)
